# revision 23
# baseline (speedup 1.0000x reference)
"""Distributed Trainium2 kernel for nn_ALEError_23742579212666.

Computes: loss = 0.7 * masked_mean((target-pred)^2, target>0)
               + 0.3 * mean(sobel(target) - sobel(pred))

Math notes:
  * sobel is linear with symmetric padding, so
    mean(sobel(t) - sobel(p)) = mean(sobel(t-p)) and the column-sum of the
    separable stencil collapses: smoothing [1,2,1] contributes a factor 4
    per axis (B,C,H -> 4^3 = 64), the derivative [-1,0,1] along W has
    column weights [-2, 0, ..., 0, +2].  Hence
      mean(sobel(d)) = 128 * sum_rows(d[..., W-1] - d[..., 0]) / N.
  * per step: m = (t>0) on DVE (4x mode), d = t-p and z = d*m on DVE
    (2x mode), sum z^2 via ScalarE Square+accum.  The mask count goes
    through TensorE (ones^T x m column sums accumulate in PSUM) and the
    PSUM fold into the accumulator runs on ScalarE (activation Copy with
    accum_out), keeping it off the DVE queue tail.  Boundary columns for
    the sobel term are column-summed by TensorE matmuls straight into a
    second PSUM region (PE is otherwise idle), so the DVE never touches
    them; the f32 head steps' boundary columns are copied by ScalarE.
  * FOLD=4 W-rows per SBUF row: SWDGE cast-DMA descriptors cover 16KB
    contiguous DRAM per partition, so a full [128, 4096] tile needs only
    128 descriptors.  Fewer descriptors matter because SDMA engine 15
    shares its AXI port with the SWDGE descriptor rings (the documented
    engines-7/15 slowdown): the profile shows engine 15 running ~16%
    slower than the other 15 and finishing the stream alone.  Halving
    descriptor traffic directly shrinks that straggle.
  * all SWDGE issues are emitted up-front on the GpSimd queue (nothing
    else runs there) and every input tile is SBUF-resident (no ring
    recycling), so the fabric is never issue-starved; the measured
    sustained rate is ~425 GB/s (16 engines at line rate).
  * every partial lands in one [128, 30] f32 tile DMA'd out whole; the
    128-partition fold and final scalar arithmetic happen on the host in
    float64.

Sharding: pure data parallel over batch, 4 images per core; per-core
partial sums are combined on the host (an on-device all-reduce has a
~20us latency floor).
"""

import sys

import numpy as np

if "/opt/trn_rl_repo" not in sys.path:
    sys.path.insert(0, "/opt/trn_rl_repo")

B, C, H, W = 32, 1, 512, 1024
NCORES = 8
BP = B // NCORES                 # batches per core
FOLD = 4                         # W-rows folded per SBUF row
RT = BP * C * H // FOLD         # 512 DRAM rows per core (folded view)
TW = W * FOLD                    # 4096
P = 128                          # SBUF partitions
NT = RT // P                     # 4 row-tiles per tensor per core
NTOT = float(B * C * H * W)      # 16777216
ALPHA = 0.3

# work-list: (row_tile, col_start, col_end).
#  - head: HWDGE f32 (fills the fabric while SWDGE warms up)
#  - middle: SWDGE bf16 cast, 2MB-read tiles
#  - tail: small tiles so the post-last-data critical path is shallow
HEAD = [(0, 0, 256), (0, 256, W)]
SW_STEPS = [(0, W, TW)]
SW_STEPS += [(i, 0, TW) for i in range(1, NT - 1)]
SW_STEPS += [(NT - 1, 0, TW // 2), (NT - 1, TW // 2, 3 * TW // 4),
             (NT - 1, 3 * TW // 4, 7 * TW // 8),
             (NT - 1, 7 * TW // 8, 15 * TW // 16), (NT - 1, 15 * TW // 16, TW)]
STEPS = HEAD + SW_STEPS
NH = len(HEAD)                   # 2
NS = len(STEPS)                  # 10

# boundary columns of the folded [RT, TW] view: (global_col, k) with
# k = 0 for W-first / 1 for W-last (per folded W-row).
BCOLS = [(j * W, 0) for j in range(FOLD)] + \
        [((j + 1) * W - 1, 1) for j in range(FOLD)]

# main accumulator-tile layout (out, [128, ACC_W]):
#   cols [0, NS)              per-step sum z^2 (128-partition partials)
#   col  NS                   mask count from the PSUM fold (partition 0)
#   col  NS+1                 last step's mask count (128 partials, DVE
#                             is_gt accum — keeps the final PSUM fold off
#                             the tail critical path)
#   cols [GH, GH+4)           head boundary-column copies (f32, 128 rows):
#                             (t,c=0), (t,c=W-1), (p,c=0), (p,c=W-1)
# second output (out2, [1, GW]): PSUM boundary column-sums, one col per
#   (step, tensor, boundary col) — every boundary matmul is atomic
#   (start=stop=True) so no PSUM accumulation chain is ever open while
#   other matmuls hit the same bank (interleaved open chains in a shared
#   bank corrupt each other).
NB = len(BCOLS)                  # 8
CNT_COL = NS
CNT2_COL = NS + 1
GH = NS + 2
ACC_W = GH + 4

# number of count matmuls (512-wide chunks; the last step counts on DVE)
TOT_CNT = sum((ce - cs + 511) // 512 for _, cs, ce in STEPS[:NS - 1])

_CACHE = {}


def _sw_bcols(cs, ce):
    """boundary cols present in a step, as (global_col, bcol_idx)."""
    return [(gc, bi) for bi, (gc, _) in enumerate(BCOLS) if cs <= gc < ce]


def _g_map():
    """Boundary-matmul plan: one psG column per (SW step, tensor, boundary
    col); adjacent boundary cols (W-1, W) merge into one 2-wide matmul.
    Returns (runs, flat): runs = [(step, ti, psg_base, col_in_step, n)],
    flat = [k-type per psG column]."""
    runs, flat = [], []
    for s in range(NH, NS):
        _, cs, ce = STEPS[s]
        present = sorted(_sw_bcols(cs, ce))
        for ti in (0, 1):
            i = 0
            while i < len(present):
                j = i
                while (j + 1 < len(present)
                       and present[j + 1][0] == present[j][0] + 1):
                    j += 1
                gc0 = present[i][0]
                n = j - i + 1
                runs.append((s, ti, len(flat), gc0 - cs, n))
                flat.extend(BCOLS[bi][1] for _, bi in present[i:j + 1])
                i = j + 1
    return runs, flat


G_RUNS, G_FLAT = _g_map()
GW = len(G_FLAT)                 # 62


def _build_nc():
    from concourse import bacc, mybir, tile

    f32 = mybir.dt.float32
    bf16 = mybir.dt.bfloat16
    Act = mybir.ActivationFunctionType
    Alu = mybir.AluOpType

    # no collectives and no partition-dependent code: build a single-core
    # NEFF (each core runs an identical independent copy) — avoids the
    # multi-core sync setup in the boot preamble.
    nc = bacc.Bacc("TRN2", target_bir_lowering=False, debug=False,
                   num_devices=1, enable_partition_id=False)
    t_ext = nc.declare_dram_parameter("target", [RT, TW], f32, isOutput=False)
    p_ext = nc.declare_dram_parameter("pred", [RT, TW], f32, isOutput=False)
    out_ext = nc.declare_dram_parameter("out", [P, ACC_W], f32, isOutput=True)
    out2_ext = nc.declare_dram_parameter("out2", [1, GW], f32, isOutput=True)

    # boundary matmuls grouped by step for emission inside the step loop
    g_by_step = {}
    for (s, ti, base, c, n) in G_RUNS:
        g_by_step.setdefault(s, []).append((ti, base, c, n))

    with tile.TileContext(nc) as tc:
        with (
            tc.tile_pool(name="io", bufs=1) as io,
            tc.tile_pool(name="mid", bufs=2) as mid,
            tc.tile_pool(name="io32", bufs=1) as io32,
            tc.tile_pool(name="one", bufs=1) as one,
            tc.tile_pool(name="ps", bufs=1, space="PSUM") as ps,
        ):
            # ---- HWDGE f32 head loads (Sync queue) ----
            head = []
            for s in range(NH):
                i, cs, ce = STEPS[s]
                t32 = io32.tile([P, ce - cs], f32, tag=f"t32_{s}")
                p32 = io32.tile([P, ce - cs], f32, tag=f"p32_{s}")
                nc.sync.dma_start(out=t32[:],
                                  in_=t_ext[P * i:P * (i + 1), cs:ce])
                nc.sync.dma_start(out=p32[:],
                                  in_=p_ext[P * i:P * (i + 1), cs:ce])
                head.append((t32, p32))

            # ---- all SWDGE cast-DMA issues, emitted up-front ----
            # (GpSimd runs nothing else; every tile is SBUF-resident so no
            # ring-recycle waits delay descriptor emission.)
            sw = {}
            for s in range(NH, NS):
                i, cs, ce = STEPS[s]
                tb = io.tile([P, ce - cs], bf16, tag=f"tb{s}")
                pb = io.tile([P, ce - cs], bf16, tag=f"pb{s}")
                nc.gpsimd.dma_start(out=tb[:],
                                    in_=t_ext[P * i:P * (i + 1), cs:ce])
                nc.gpsimd.dma_start(out=pb[:],
                                    in_=p_ext[P * i:P * (i + 1), cs:ce])
                sw[s] = (tb, pb)

            acc = one.tile([P, ACC_W], f32)
            acc2 = one.tile([1, GW], f32)
            ones_b = one.tile([P, 1], bf16)
            nc.vector.memset(ones_b[:], 1.0)
            psN = ps.tile([1, 512], f32)       # mask-count column sums
            psG = ps.tile([1, GW], f32)        # boundary column sums
            psD = ps.tile([1, 512], f32)       # dummy-matmul sink
            foldN = one.tile([1, 512], f32)    # throwaway fold output

            n_cnt = 0
            warm = None
            for s, (i, cs, ce) in enumerate(STEPS):
                wdt = ce - cs
                if s < NH:
                    tb, pb = head[s]
                else:
                    tb, pb = sw[s]

                # the last steps' tiles come from the persistent pool so
                # they never wait on a mid-pool buffer release
                pool = one if s >= NS - 4 else mid
                sfx = str(s) if s >= NS - 4 else ""
                m = pool.tile([P, wdt], bf16, tag="m" + sfx)
                d = pool.tile([P, wdt], bf16, tag="d" + sfx)
                z = pool.tile([P, wdt], bf16, tag="z" + sfx)
                sj = pool.tile([P, wdt], bf16, tag="sj" + sfx)
                if s == NS - 2:
                    warm = (sj, wdt)

                # mask first: only needs tb, can run while pb is in flight.
                # The last step's mask column-sums on DVE (1x with accum)
                # so the count never touches PSUM on the tail.
                if s == NS - 1:
                    nc.vector.tensor_scalar(m[:], tb[:], 0.0, 1.0,
                                            Alu.is_gt, Alu.mult,
                                            accum_out=acc[:, CNT2_COL:GH])
                else:
                    nc.vector.tensor_scalar(m[:], tb[:], 0.0, None, Alu.is_gt)
                nc.vector.tensor_tensor(d[:], tb[:], pb[:], Alu.subtract)
                nc.vector.tensor_tensor(z[:], d[:], m[:], Alu.mult)

                if s >= NS - 2:
                    # square+accum on DVE for the tail steps: ScalarE would
                    # serialize behind earlier accum reads right on the
                    # tail critical path
                    nc.vector.scalar_tensor_tensor(sj[:], z[:], 1.0, z[:],
                                                   Alu.mult, Alu.mult,
                                                   accum_out=acc[:, s:s + 1])
                else:
                    nc.scalar.activation(sj[:], z[:], Act.Square,
                                         accum_out=acc[:, s:s + 1])

                if s < NH:
                    # head boundary columns -> ScalarE copies (f32, exact)
                    for gc, bi in _sw_bcols(cs, ce):
                        c = gc - cs
                        for ti, buf in ((0, tb), (1, pb)):
                            col = GH + 2 * ti + (0 if bi == 0 else 1)
                            nc.scalar.activation(acc[:, col:col + 1],
                                                 buf[:, c:c + 1], Act.Copy)
                else:
                    # SW boundary columns -> TensorE column sums in PSUM
                    # (atomic: start=stop=True per matmul)
                    for ti, base, c, n in g_by_step.get(s, ()):
                        buf = tb if ti == 0 else pb
                        nc.tensor.matmul(psG[0:1, base:base + n], ones_b[:],
                                         buf[:, c:c + n],
                                         start=True, stop=True)

                # mask count: ones^T x m column sums accumulate in PSUM
                # (the last step's count went through the DVE accum above)
                if s < NS - 1:
                    for k in range(0, wdt, 512):
                        kw = min(512, wdt - k)
                        nc.tensor.matmul(psN[0:1, 0:kw], ones_b[:],
                                         m[:, k:k + kw],
                                         start=(n_cnt == 0),
                                         stop=(n_cnt == TOT_CNT - 1))
                        n_cnt += 1
            assert n_cnt == TOT_CNT

            # fold PSUM into the accumulator tiles on ScalarE (off the DVE
            # queue): count = sum over psN, boundary sums copied verbatim
            nc.scalar.activation(foldN[0:1, :], psN[:], Act.Copy,
                                 accum_out=acc[0:1, CNT_COL:CNT_COL + 1])
            nc.scalar.activation(acc2[0:1, :], psG[:], Act.Copy)

            # dummy atomic matmuls keep the PE HAM clock-gate released
            # through the NEFF's per-semaphore reset epilogue (the PE
            # sequencer executes ~52 resets; at K=4/8 they run 2x slower).
            # They read a persistent tail tile and write an unread PSUM
            # bank, finishing before the output DMA completes.
            warm_t, warm_w = warm
            for _ in range(9):
                nc.tensor.matmul(psD[0:1, 0:warm_w], ones_b[:],
                                 warm_t[:], start=True, stop=True)

            nc.sync.dma_start(out=out_ext[:, :], in_=acc[:])
            nc.sync.dma_start(out=out2_ext[:, :], in_=acc2[:])

    nc.compile()
    return nc


def get_nc():
    if "nc" not in _CACHE:
        _CACHE["nc"] = _build_nc()
    return _CACHE["nc"]


def make_in_maps(pred, target):
    pred = np.ascontiguousarray(np.asarray(pred, dtype=np.float32))
    target = np.ascontiguousarray(np.asarray(target, dtype=np.float32))
    in_maps = []
    for c in range(NCORES):
        in_maps.append({
            "pred": pred[c * BP:(c + 1) * BP].reshape(RT, TW),
            "target": target[c * BP:(c + 1) * BP].reshape(RT, TW),
        })
    return in_maps


def _gw(ti, k):
    """G-term weight for tensor ti (0=target, 1=pred) at a k-type column."""
    return 1.0 if (k == 1) != (ti == 1) else -1.0


def combine(results):
    """results: list (per core) of {"out": (128, ACC_W) f32} -> scalar."""
    S = NV = G = 0.0
    for c in range(NCORES):
        v = np.asarray(results[c]["out"], dtype=np.float64)
        v2 = np.asarray(results[c]["out2"], dtype=np.float64)
        S += v[:, 0:NS].sum()
        NV += v[0, CNT_COL] + v[:, CNT2_COL].sum()
        # head copies: (t,c0), (t,cW-1), (p,c0), (p,cW-1) as 128-partials
        for ti in (0, 1):
            G += _gw(ti, 0) * v[:, GH + 2 * ti].sum()
            G += _gw(ti, 1) * v[:, GH + 2 * ti + 1].sum()
        # PSUM boundary sums: one col per (step, tensor, boundary col)
        for (s, ti, base, c, n) in G_RUNS:
            for j in range(n):
                G += _gw(ti, G_FLAT[base + j]) * v2[0, base + j]
    loss = (1.0 - ALPHA) * (S / NV) + ALPHA * 128.0 * G / NTOT
    return np.asarray(loss, dtype=np.float32)


def kernel(pred, target):
    from concourse.bass_utils import run_bass_kernel_spmd

    nc = get_nc()
    in_maps = make_in_maps(pred, target)
    res = run_bass_kernel_spmd(nc, in_maps, core_ids=list(range(NCORES)))
    return combine(res.results)


# revision 25
# speedup vs baseline: 1.0049x; 1.0049x over previous
"""Distributed Trainium2 kernel for nn_ALEError_23742579212666.

Computes: loss = 0.7 * masked_mean((target-pred)^2, target>0)
               + 0.3 * mean(sobel(target) - sobel(pred))

Math notes:
  * sobel is linear with symmetric padding, so
    mean(sobel(t) - sobel(p)) = mean(sobel(t-p)) and the column-sum of the
    separable stencil collapses: smoothing [1,2,1] contributes a factor 4
    per axis (B,C,H -> 4^3 = 64), the derivative [-1,0,1] along W has
    column weights [-2, 0, ..., 0, +2].  Hence
      mean(sobel(d)) = 128 * sum_rows(d[..., W-1] - d[..., 0]) / N.
  * per step: m = (t>0) on DVE (4x mode), d = t-p and z = d*m on DVE
    (2x mode), sum z^2 via ScalarE Square+accum.  The mask count goes
    through TensorE (ones^T x m column sums accumulate in PSUM) and the
    PSUM fold into the accumulator runs on ScalarE (activation Copy with
    accum_out), keeping it off the DVE queue tail.  Boundary columns for
    the sobel term are column-summed by TensorE matmuls straight into a
    second PSUM region (PE is otherwise idle), so the DVE never touches
    them; the f32 head steps' boundary columns are copied by ScalarE.
  * FOLD=4 W-rows per SBUF row: SWDGE cast-DMA descriptors cover 16KB
    contiguous DRAM per partition, so a full [128, 4096] tile needs only
    128 descriptors.  Fewer descriptors matter because SDMA engine 15
    shares its AXI port with the SWDGE descriptor rings (the documented
    engines-7/15 slowdown): the profile shows engine 15 running ~16%
    slower than the other 15 and finishing the stream alone.  Halving
    descriptor traffic directly shrinks that straggle.
  * all SWDGE issues are emitted up-front on the GpSimd queue (nothing
    else runs there) and every input tile is SBUF-resident (no ring
    recycling), so the fabric is never issue-starved; the measured
    sustained rate is ~425 GB/s (16 engines at line rate).
  * every partial lands in one [128, 30] f32 tile DMA'd out whole; the
    128-partition fold and final scalar arithmetic happen on the host in
    float64.

Sharding: pure data parallel over batch, 4 images per core; per-core
partial sums are combined on the host (an on-device all-reduce has a
~20us latency floor).
"""

import sys

import numpy as np

if "/opt/trn_rl_repo" not in sys.path:
    sys.path.insert(0, "/opt/trn_rl_repo")

B, C, H, W = 32, 1, 512, 1024
NCORES = 8
BP = B // NCORES                 # batches per core
FOLD = 4                         # W-rows folded per SBUF row
RT = BP * C * H // FOLD         # 512 DRAM rows per core (folded view)
TW = W * FOLD                    # 4096
P = 128                          # SBUF partitions
NT = RT // P                     # 4 row-tiles per tensor per core
NTOT = float(B * C * H * W)      # 16777216
ALPHA = 0.3

# work-list: (row_tile, col_start, col_end).
#  - head: HWDGE f32 (fills the fabric while SWDGE warms up)
#  - middle: SWDGE bf16 cast, 2MB-read tiles
#  - tail: small tiles so the post-last-data critical path is shallow
HEAD = [(0, 0, 256), (0, 256, W)]
SW_STEPS = [(0, W, TW)]
SW_STEPS += [(i, 0, TW) for i in range(1, NT - 1)]
SW_STEPS += [(NT - 1, 0, TW // 2), (NT - 1, TW // 2, 3 * TW // 4),
             (NT - 1, 3 * TW // 4, 7 * TW // 8),
             (NT - 1, 7 * TW // 8, 15 * TW // 16), (NT - 1, 15 * TW // 16, TW)]
STEPS = HEAD + SW_STEPS
NH = len(HEAD)                   # 2
NS = len(STEPS)                  # 10

# boundary columns of the folded [RT, TW] view: (global_col, k) with
# k = 0 for W-first / 1 for W-last (per folded W-row).
BCOLS = [(j * W, 0) for j in range(FOLD)] + \
        [((j + 1) * W - 1, 1) for j in range(FOLD)]

# main accumulator-tile layout (out, [128, ACC_W]):
#   cols [0, NS)              per-step sum z^2 (128-partition partials)
#   col  NS                   mask count from the PSUM fold (partition 0)
#   col  NS+1                 last step's mask count (128 partials, DVE
#                             is_gt accum — keeps the final PSUM fold off
#                             the tail critical path)
#   cols [GH, GH+4)           head boundary-column copies (f32, 128 rows):
#                             (t,c=0), (t,c=W-1), (p,c=0), (p,c=W-1)
# second output (out2, [1, GW]): PSUM boundary column-sums, one col per
#   (step, tensor, boundary col) — every boundary matmul is atomic
#   (start=stop=True) so no PSUM accumulation chain is ever open while
#   other matmuls hit the same bank (interleaved open chains in a shared
#   bank corrupt each other).
NB = len(BCOLS)                  # 8
CNT_COL = NS
CNT2_COL = NS + 1
GH = NS + 2
ACC_W = GH + 4

# number of count matmuls (512-wide chunks; the last step counts on DVE)
TOT_CNT = sum((ce - cs + 511) // 512 for _, cs, ce in STEPS[:NS - 1])

_CACHE = {}


def _sw_bcols(cs, ce):
    """boundary cols present in a step, as (global_col, bcol_idx)."""
    return [(gc, bi) for bi, (gc, _) in enumerate(BCOLS) if cs <= gc < ce]


def _g_map():
    """Boundary-matmul plan: one psG column per (SW step, tensor, boundary
    col); adjacent boundary cols (W-1, W) merge into one 2-wide matmul.
    Returns (runs, flat): runs = [(step, ti, psg_base, col_in_step, n)],
    flat = [k-type per psG column]."""
    runs, flat = [], []
    for s in range(NH, NS):
        _, cs, ce = STEPS[s]
        present = sorted(_sw_bcols(cs, ce))
        for ti in (0, 1):
            i = 0
            while i < len(present):
                j = i
                while (j + 1 < len(present)
                       and present[j + 1][0] == present[j][0] + 1):
                    j += 1
                gc0 = present[i][0]
                n = j - i + 1
                runs.append((s, ti, len(flat), gc0 - cs, n))
                flat.extend(BCOLS[bi][1] for _, bi in present[i:j + 1])
                i = j + 1
    return runs, flat


G_RUNS, G_FLAT = _g_map()
GW = len(G_FLAT)                 # 62


def _build_nc():
    from concourse import bacc, mybir, tile

    f32 = mybir.dt.float32
    bf16 = mybir.dt.bfloat16
    Act = mybir.ActivationFunctionType
    Alu = mybir.AluOpType

    # no collectives and no partition-dependent code: build a single-core
    # NEFF (each core runs an identical independent copy) — avoids the
    # multi-core sync setup in the boot preamble.
    nc = bacc.Bacc("TRN2", target_bir_lowering=False, debug=False,
                   num_devices=1, enable_partition_id=False)
    t_ext = nc.declare_dram_parameter("target", [RT, TW], f32, isOutput=False)
    p_ext = nc.declare_dram_parameter("pred", [RT, TW], f32, isOutput=False)
    out_ext = nc.declare_dram_parameter("out", [P, ACC_W], f32, isOutput=True)
    out2_ext = nc.declare_dram_parameter("out2", [1, GW], f32, isOutput=True)

    # boundary matmuls grouped by step for emission inside the step loop
    g_by_step = {}
    for (s, ti, base, c, n) in G_RUNS:
        g_by_step.setdefault(s, []).append((ti, base, c, n))

    with tile.TileContext(nc) as tc:
        with (
            tc.tile_pool(name="io", bufs=1) as io,
            tc.tile_pool(name="mid", bufs=2) as mid,
            tc.tile_pool(name="io32", bufs=1) as io32,
            tc.tile_pool(name="one", bufs=1) as one,
            tc.tile_pool(name="ps", bufs=1, space="PSUM") as ps,
        ):
            # ---- HWDGE f32 head loads (Sync queue) ----
            head = []
            for s in range(NH):
                i, cs, ce = STEPS[s]
                t32 = io32.tile([P, ce - cs], f32, tag=f"t32_{s}")
                p32 = io32.tile([P, ce - cs], f32, tag=f"p32_{s}")
                nc.sync.dma_start(out=t32[:],
                                  in_=t_ext[P * i:P * (i + 1), cs:ce])
                nc.sync.dma_start(out=p32[:],
                                  in_=p_ext[P * i:P * (i + 1), cs:ce])
                head.append((t32, p32))

            # ---- all SWDGE cast-DMA issues, emitted up-front ----
            # (GpSimd runs nothing else; every tile is SBUF-resident so no
            # ring-recycle waits delay descriptor emission.)
            sw = {}
            for s in range(NH, NS):
                i, cs, ce = STEPS[s]
                tb = io.tile([P, ce - cs], bf16, tag=f"tb{s}")
                pb = io.tile([P, ce - cs], bf16, tag=f"pb{s}")
                nc.gpsimd.dma_start(out=tb[:],
                                    in_=t_ext[P * i:P * (i + 1), cs:ce])
                nc.gpsimd.dma_start(out=pb[:],
                                    in_=p_ext[P * i:P * (i + 1), cs:ce])
                sw[s] = (tb, pb)

            acc = one.tile([P, ACC_W], f32)
            acc2 = one.tile([1, GW], f32)
            ones_b = one.tile([P, 1], bf16)
            nc.vector.memset(ones_b[:], 1.0)
            last_w = STEPS[NS - 1][2] - STEPS[NS - 1][1]
            ones_w = one.tile([P, last_w], bf16)
            nc.vector.memset(ones_w[:], 1.0)
            psN = ps.tile([1, 512], f32)       # mask-count column sums
            psG = ps.tile([1, GW], f32)        # boundary column sums
            psD = ps.tile([1, 512], f32)       # dummy-matmul sink
            foldN = one.tile([1, 512], f32)    # throwaway fold output

            n_cnt = 0
            warm = None
            for s, (i, cs, ce) in enumerate(STEPS):
                wdt = ce - cs
                if s < NH:
                    tb, pb = head[s]
                else:
                    tb, pb = sw[s]

                # the last steps' tiles come from the persistent pool so
                # they never wait on a mid-pool buffer release
                pool = one if s >= NS - 4 else mid
                sfx = str(s) if s >= NS - 4 else ""
                m = pool.tile([P, wdt], bf16, tag="m" + sfx)
                d = pool.tile([P, wdt], bf16, tag="d" + sfx)
                z = pool.tile([P, wdt], bf16, tag="z" + sfx)
                sj = pool.tile([P, wdt], bf16, tag="sj" + sfx)
                if s == NS - 2:
                    warm = (sj, wdt)

                # mask first: only needs tb, can run while pb is in flight.
                # The last step's mask column-sums on DVE (1x with accum)
                # so the count never touches PSUM on the tail.
                if s == NS - 1:
                    nc.vector.scalar_tensor_tensor(m[:], tb[:], 0.0,
                                                   ones_w[:], Alu.is_gt,
                                                   Alu.mult,
                                                   accum_out=acc[:, CNT2_COL:GH])
                else:
                    nc.vector.tensor_scalar(m[:], tb[:], 0.0, None, Alu.is_gt)
                nc.vector.tensor_tensor(d[:], tb[:], pb[:], Alu.subtract)
                nc.vector.tensor_tensor(z[:], d[:], m[:], Alu.mult)

                if s >= NS - 2:
                    # square+accum on DVE for the tail steps: ScalarE would
                    # serialize behind earlier accum reads right on the
                    # tail critical path
                    nc.vector.scalar_tensor_tensor(sj[:], z[:], 1.0, z[:],
                                                   Alu.mult, Alu.mult,
                                                   accum_out=acc[:, s:s + 1])
                else:
                    nc.scalar.activation(sj[:], z[:], Act.Square,
                                         accum_out=acc[:, s:s + 1])

                if s < NH:
                    # head boundary columns -> ScalarE copies (f32, exact)
                    for gc, bi in _sw_bcols(cs, ce):
                        c = gc - cs
                        for ti, buf in ((0, tb), (1, pb)):
                            col = GH + 2 * ti + (0 if bi == 0 else 1)
                            nc.scalar.activation(acc[:, col:col + 1],
                                                 buf[:, c:c + 1], Act.Copy)
                else:
                    # SW boundary columns -> TensorE column sums in PSUM
                    # (atomic: start=stop=True per matmul)
                    for ti, base, c, n in g_by_step.get(s, ()):
                        buf = tb if ti == 0 else pb
                        nc.tensor.matmul(psG[0:1, base:base + n], ones_b[:],
                                         buf[:, c:c + n],
                                         start=True, stop=True)

                # mask count: ones^T x m column sums accumulate in PSUM
                # (the last step's count went through the DVE accum above)
                if s < NS - 1:
                    for k in range(0, wdt, 512):
                        kw = min(512, wdt - k)
                        nc.tensor.matmul(psN[0:1, 0:kw], ones_b[:],
                                         m[:, k:k + kw],
                                         start=(n_cnt == 0),
                                         stop=(n_cnt == TOT_CNT - 1))
                        n_cnt += 1
            assert n_cnt == TOT_CNT

            # fold PSUM into the accumulator tiles on ScalarE (off the DVE
            # queue): count = sum over psN, boundary sums copied verbatim
            nc.scalar.activation(foldN[0:1, :], psN[:], Act.Copy,
                                 accum_out=acc[0:1, CNT_COL:CNT_COL + 1])
            nc.scalar.activation(acc2[0:1, :], psG[:], Act.Copy)

            # dummy atomic matmuls keep the PE HAM clock-gate released
            # through the NEFF's per-semaphore reset epilogue (the PE
            # sequencer executes ~52 resets; at K=4/8 they run 2x slower).
            # They read a persistent tail tile and write an unread PSUM
            # bank, finishing before the output DMA completes.
            warm_t, warm_w = warm
            for _ in range(9):
                nc.tensor.matmul(psD[0:1, 0:warm_w], ones_b[:],
                                 warm_t[:], start=True, stop=True)

            nc.sync.dma_start(out=out_ext[:, :], in_=acc[:])
            nc.sync.dma_start(out=out2_ext[:, :], in_=acc2[:])

    nc.compile()
    return nc


def get_nc():
    if "nc" not in _CACHE:
        _CACHE["nc"] = _build_nc()
    return _CACHE["nc"]


def make_in_maps(pred, target):
    pred = np.ascontiguousarray(np.asarray(pred, dtype=np.float32))
    target = np.ascontiguousarray(np.asarray(target, dtype=np.float32))
    in_maps = []
    for c in range(NCORES):
        in_maps.append({
            "pred": pred[c * BP:(c + 1) * BP].reshape(RT, TW),
            "target": target[c * BP:(c + 1) * BP].reshape(RT, TW),
        })
    return in_maps


def _gw(ti, k):
    """G-term weight for tensor ti (0=target, 1=pred) at a k-type column."""
    return 1.0 if (k == 1) != (ti == 1) else -1.0


def combine(results):
    """results: list (per core) of {"out": (128, ACC_W) f32} -> scalar."""
    S = NV = G = 0.0
    for c in range(NCORES):
        v = np.asarray(results[c]["out"], dtype=np.float64)
        v2 = np.asarray(results[c]["out2"], dtype=np.float64)
        S += v[:, 0:NS].sum()
        NV += v[0, CNT_COL] + v[:, CNT2_COL].sum()
        # head copies: (t,c0), (t,cW-1), (p,c0), (p,cW-1) as 128-partials
        for ti in (0, 1):
            G += _gw(ti, 0) * v[:, GH + 2 * ti].sum()
            G += _gw(ti, 1) * v[:, GH + 2 * ti + 1].sum()
        # PSUM boundary sums: one col per (step, tensor, boundary col)
        for (s, ti, base, c, n) in G_RUNS:
            for j in range(n):
                G += _gw(ti, G_FLAT[base + j]) * v2[0, base + j]
    loss = (1.0 - ALPHA) * (S / NV) + ALPHA * 128.0 * G / NTOT
    return np.asarray(loss, dtype=np.float32)


def kernel(pred, target):
    from concourse.bass_utils import run_bass_kernel_spmd

    nc = get_nc()
    in_maps = make_in_maps(pred, target)
    res = run_bass_kernel_spmd(nc, in_maps, core_ids=list(range(NCORES)))
    return combine(res.results)


# revision 29
# speedup vs baseline: 1.0330x; 1.0279x over previous
"""Distributed Trainium2 kernel for nn_ALEError_23742579212666.

Computes: loss = 0.7 * masked_mean((target-pred)^2, target>0)
               + 0.3 * mean(sobel(target) - sobel(pred))

Math notes:
  * sobel is linear with symmetric padding, so
    mean(sobel(t) - sobel(p)) = mean(sobel(t-p)) and the column-sum of the
    separable stencil collapses: smoothing [1,2,1] contributes a factor 4
    per axis (B,C,H -> 4^3 = 64), the derivative [-1,0,1] along W has
    column weights [-2, 0, ..., 0, +2].  Hence
      mean(sobel(d)) = 128 * sum_rows(d[..., W-1] - d[..., 0]) / N.
  * per step: m = (t>0) on DVE (4x mode), d = t-p and z = d*m on DVE
    (2x mode), sum z^2 via ScalarE Square+accum.  The mask count goes
    through TensorE (ones^T x m column sums accumulate in PSUM) and the
    PSUM fold runs on ScalarE (activation Copy with accum_out), keeping
    it off the DVE queue tail.  Boundary columns for the sobel term are
    column-summed by TensorE matmuls straight into a second PSUM region
    (PE is otherwise idle), atomically (start=stop=True per matmul):
    interleaved OPEN accumulation chains in a shared PSUM bank corrupt
    each other, atomic writes don't.  The f32 head steps' boundary
    columns are copied by ScalarE.
  * FOLD=4 W-rows per SBUF row: SWDGE cast-DMA descriptors cover 16KB
    contiguous DRAM per partition, so a full [128, 4096] tile needs only
    128 descriptors.  Fewer descriptors matter because SDMA engine 15
    shares its AXI port with the SWDGE descriptor rings (the documented
    engines-7/15 slowdown): with FOLD=2 the profile showed engine 15
    ~16% slower than the rest, finishing the stream alone ~9us late;
    halving descriptor traffic roughly halves that straggle.
  * all SWDGE issues are emitted up-front on the GpSimd queue (nothing
    else runs there) and every input tile is SBUF-resident (no ring
    recycling), so the fabric is never issue-starved; the measured
    sustained rate is ~425 GB/s (16 engines at line rate).
  * two outputs, issued from the two independent HWDGE queues so their
    completions overlap: `out` [128, NS+4] (per-step z^2 partials + f32
    head boundary copies) goes out on Sync as soon as the DVE tail
    drains; `out2` [1, 1+GW] (count fold + boundary column sums) goes
    out on the Scalar queue right after the PSUM folds.  Host combines
    in float64.

Sharding: pure data parallel over batch, 4 images per core; per-core
partial sums are combined on the host (an on-device all-reduce has a
~20us latency floor).
"""

import sys

import numpy as np

if "/opt/trn_rl_repo" not in sys.path:
    sys.path.insert(0, "/opt/trn_rl_repo")

B, C, H, W = 32, 1, 512, 1024
NCORES = 8
BP = B // NCORES                 # batches per core
FOLD = 4                         # W-rows folded per SBUF row
RT = BP * C * H // FOLD          # 512 DRAM rows per core (folded view)
TW = W * FOLD                    # 4096
P = 128                          # SBUF partitions
NT = RT // P                     # 4 row-tiles per tensor per core
NTOT = float(B * C * H * W)      # 16777216
ALPHA = 0.3

# work-list: (row_tile, col_start, col_end).
#  - head: HWDGE f32 (fills the fabric while SWDGE warms up; kept small
#    because DVE ops on f32 run at 1x)
#  - middle: SWDGE bf16 cast, 2MB-read tiles
#  - tail: small tiles so the post-last-data critical path is shallow
HEAD = [(0, 0, 256), (0, 256, 512)]
SW_STEPS = [(0, 512, TW)]
SW_STEPS += [(i, 0, TW) for i in range(1, NT - 1)]
SW_STEPS += [(NT - 1, 0, TW // 2), (NT - 1, TW // 2, 3 * TW // 4),
             (NT - 1, 3 * TW // 4, 7 * TW // 8), (NT - 1, 7 * TW // 8, TW)]
STEPS = HEAD + SW_STEPS
NH = len(HEAD)                   # 2
NS = len(STEPS)                  # 9

# boundary columns of the folded [RT, TW] view: (global_col, k) with
# k = 0 for W-first / 1 for W-last (per folded W-row).
BCOLS = [(j * W, 0) for j in range(FOLD)] + \
        [((j + 1) * W - 1, 1) for j in range(FOLD)]
NB = len(BCOLS)                  # 8

# boundary cols covered by the f32 head (the rest go through SW steps):
# HB = [(step, col_in_step, k)], copies laid out as (ti-major, HB-minor)
HB = [(s, gc - cs, k)
      for s, (i, cs, ce) in enumerate(HEAD)
      for (gc, k) in BCOLS if cs <= gc < ce]

# main output (out, [128, ACC_W]):
#   cols [0, NS)          per-step sum z^2 (128-partition partials)
#   cols [GH, GH+2*|HB|)  head boundary-column copies (f32, 128 rows)
# second output (out2, [1, 1+GW]):
#   col 0           mask count (PSUM fold)
#   cols [1, 1+GW)  boundary column sums, one per (step, tensor, col)
GH = NS
ACC_W = GH + 2 * len(HB)

# number of count matmuls (512-wide chunks)
TOT_CNT = sum((ce - cs + 511) // 512 for _, cs, ce in STEPS)

_CACHE = {}


def _sw_bcols(cs, ce):
    """boundary cols present in a step, as (global_col, bcol_idx)."""
    return [(gc, bi) for bi, (gc, _) in enumerate(BCOLS) if cs <= gc < ce]


def _g_map():
    """Boundary-matmul plan: one psG column per (SW step, tensor, boundary
    col); adjacent boundary cols (W-1, W) merge into one 2-wide matmul.
    Returns (runs, flat): runs = [(step, ti, psg_base, col_in_step, n)],
    flat = [k-type per psG column]."""
    runs, flat = [], []
    for s in range(NH, NS):
        _, cs, ce = STEPS[s]
        present = sorted(_sw_bcols(cs, ce))
        for ti in (0, 1):
            i = 0
            while i < len(present):
                j = i
                while (j + 1 < len(present)
                       and present[j + 1][0] == present[j][0] + 1):
                    j += 1
                gc0 = present[i][0]
                n = j - i + 1
                runs.append((s, ti, len(flat), gc0 - cs, n))
                flat.extend(BCOLS[bi][1] for _, bi in present[i:j + 1])
                i = j + 1
    return runs, flat


G_RUNS, G_FLAT = _g_map()
GW = len(G_FLAT)


def _build_nc():
    from concourse import bacc, mybir, tile

    f32 = mybir.dt.float32
    bf16 = mybir.dt.bfloat16
    Act = mybir.ActivationFunctionType
    Alu = mybir.AluOpType

    # no collectives and no partition-dependent code: build a single-core
    # NEFF (each core runs an identical independent copy) — avoids the
    # multi-core sync setup in the boot preamble.
    nc = bacc.Bacc("TRN2", target_bir_lowering=False, debug=False,
                   num_devices=1, enable_partition_id=False)
    t_ext = nc.declare_dram_parameter("target", [RT, TW], f32, isOutput=False)
    p_ext = nc.declare_dram_parameter("pred", [RT, TW], f32, isOutput=False)
    out_ext = nc.declare_dram_parameter("out", [P, ACC_W], f32, isOutput=True)
    out2_ext = nc.declare_dram_parameter("out2", [1, 1 + GW], f32,
                                         isOutput=True)

    # boundary matmuls grouped by step for emission inside the step loop
    g_by_step = {}
    for (s, ti, base, c, n) in G_RUNS:
        g_by_step.setdefault(s, []).append((ti, base, c, n))

    with tile.TileContext(nc) as tc:
        with (
            tc.tile_pool(name="io", bufs=1) as io,
            tc.tile_pool(name="mid", bufs=2) as mid,
            tc.tile_pool(name="io32", bufs=1) as io32,
            tc.tile_pool(name="one", bufs=1) as one,
            tc.tile_pool(name="ps", bufs=1, space="PSUM") as ps,
        ):
            # ---- HWDGE f32 head loads (Sync queue) ----
            head = []
            for s in range(NH):
                i, cs, ce = STEPS[s]
                t32 = io32.tile([P, ce - cs], f32, tag=f"t32_{s}")
                p32 = io32.tile([P, ce - cs], f32, tag=f"p32_{s}")
                nc.sync.dma_start(out=t32[:],
                                  in_=t_ext[P * i:P * (i + 1), cs:ce])
                nc.sync.dma_start(out=p32[:],
                                  in_=p_ext[P * i:P * (i + 1), cs:ce])
                head.append((t32, p32))

            # ---- all SWDGE cast-DMA issues, emitted up-front ----
            # (GpSimd runs nothing else; every tile is SBUF-resident so no
            # ring-recycle waits delay descriptor emission.)
            sw = {}
            for s in range(NH, NS):
                i, cs, ce = STEPS[s]
                tb = io.tile([P, ce - cs], bf16, tag=f"tb{s}")
                pb = io.tile([P, ce - cs], bf16, tag=f"pb{s}")
                nc.gpsimd.dma_start(out=tb[:],
                                    in_=t_ext[P * i:P * (i + 1), cs:ce])
                nc.gpsimd.dma_start(out=pb[:],
                                    in_=p_ext[P * i:P * (i + 1), cs:ce])
                sw[s] = (tb, pb)

            acc = one.tile([P, ACC_W], f32)
            acc2 = one.tile([1, 1 + GW], f32)
            ones_b = one.tile([P, 1], bf16)
            nc.vector.memset(ones_b[:], 1.0)
            psN = ps.tile([1, 512], f32)       # mask-count column sums
            psG = ps.tile([1, GW], f32)        # boundary column sums
            foldN = one.tile([1, 512], f32)    # throwaway fold output

            n_cnt = 0
            for s, (i, cs, ce) in enumerate(STEPS):
                wdt = ce - cs
                if s < NH:
                    tb, pb = head[s]
                else:
                    tb, pb = sw[s]

                # the last steps' tiles come from the persistent pool so
                # they never wait on a mid-pool buffer release
                pool = one if s >= NS - 3 else mid
                sfx = str(s) if s >= NS - 3 else ""
                m = pool.tile([P, wdt], bf16, tag="m" + sfx)
                d = pool.tile([P, wdt], bf16, tag="d" + sfx)
                z = pool.tile([P, wdt], bf16, tag="z" + sfx)
                sj = pool.tile([P, wdt], bf16, tag="sj" + sfx)

                # mask first: only needs tb, can run while pb is in flight
                nc.vector.tensor_scalar(m[:], tb[:], 0.0, None, Alu.is_gt)
                nc.vector.tensor_tensor(d[:], tb[:], pb[:], Alu.subtract)
                nc.vector.tensor_tensor(z[:], d[:], m[:], Alu.mult)

                if s >= NS - 2:
                    # square+accum on DVE for the tail steps: ScalarE would
                    # serialize behind earlier accum reads right on the
                    # tail critical path
                    nc.vector.scalar_tensor_tensor(sj[:], z[:], 1.0, z[:],
                                                   Alu.mult, Alu.mult,
                                                   accum_out=acc[:, s:s + 1])
                else:
                    nc.scalar.activation(sj[:], z[:], Act.Square,
                                         accum_out=acc[:, s:s + 1])

                if s < NH:
                    # head boundary columns -> ScalarE copies (f32, exact)
                    for hi, (hs, c, k) in enumerate(HB):
                        if hs != s:
                            continue
                        for ti, buf in ((0, tb), (1, pb)):
                            col = GH + len(HB) * ti + hi
                            nc.scalar.activation(acc[:, col:col + 1],
                                                 buf[:, c:c + 1], Act.Copy)
                else:
                    # SW boundary columns -> TensorE column sums in PSUM
                    # (atomic: start=stop=True per matmul)
                    for ti, base, c, n in g_by_step.get(s, ()):
                        buf = tb if ti == 0 else pb
                        nc.tensor.matmul(psG[0:1, base:base + n], ones_b[:],
                                         buf[:, c:c + n],
                                         start=True, stop=True)

                # mask count: ones^T x m column sums accumulate in PSUM
                for k in range(0, wdt, 512):
                    kw = min(512, wdt - k)
                    nc.tensor.matmul(psN[0:1, 0:kw], ones_b[:],
                                     m[:, k:k + kw],
                                     start=(n_cnt == 0),
                                     stop=(n_cnt == TOT_CNT - 1))
                    n_cnt += 1
            assert n_cnt == TOT_CNT

            # fold PSUM into acc2 on ScalarE (off the DVE queue): count =
            # sum over psN, boundary sums copied verbatim
            nc.scalar.activation(foldN[0:1, :], psN[:], Act.Copy,
                                 accum_out=acc2[0:1, 0:1])
            nc.scalar.activation(acc2[0:1, 1:1 + GW], psG[:], Act.Copy)

            # two HWDGE queues so the output DMAs overlap: acc as soon as
            # the DVE tail drains (Sync), acc2 right after the folds (ACT)
            nc.sync.dma_start(out=out_ext[:, :], in_=acc[:])
            nc.scalar.dma_start(out=out2_ext[:, :], in_=acc2[:])

    nc.compile()
    return nc


def get_nc():
    if "nc" not in _CACHE:
        _CACHE["nc"] = _build_nc()
    return _CACHE["nc"]


def make_in_maps(pred, target):
    pred = np.ascontiguousarray(np.asarray(pred, dtype=np.float32))
    target = np.ascontiguousarray(np.asarray(target, dtype=np.float32))
    in_maps = []
    for c in range(NCORES):
        in_maps.append({
            "pred": pred[c * BP:(c + 1) * BP].reshape(RT, TW),
            "target": target[c * BP:(c + 1) * BP].reshape(RT, TW),
        })
    return in_maps


def _gw(ti, k):
    """G-term weight for tensor ti (0=target, 1=pred) at a k-type column."""
    return 1.0 if (k == 1) != (ti == 1) else -1.0


def combine(results):
    """results: list (per core) of {"out", "out2"} -> scalar loss."""
    S = NV = G = 0.0
    for c in range(NCORES):
        v = np.asarray(results[c]["out"], dtype=np.float64)
        v2 = np.asarray(results[c]["out2"], dtype=np.float64)
        S += v[:, 0:NS].sum()
        NV += v2[0, 0]
        # head boundary copies as 128-partials
        for ti in (0, 1):
            for hi, (hs, _, k) in enumerate(HB):
                G += _gw(ti, k) * v[:, GH + len(HB) * ti + hi].sum()
        # PSUM boundary sums: one col per (step, tensor, boundary col)
        for (s, ti, base, c, n) in G_RUNS:
            for j in range(n):
                G += _gw(ti, G_FLAT[base + j]) * v2[0, 1 + base + j]
    loss = (1.0 - ALPHA) * (S / NV) + ALPHA * 128.0 * G / NTOT
    return np.asarray(loss, dtype=np.float32)


def kernel(pred, target):
    from concourse.bass_utils import run_bass_kernel_spmd

    nc = get_nc()
    in_maps = make_in_maps(pred, target)
    res = run_bass_kernel_spmd(nc, in_maps, core_ids=list(range(NCORES)))
    return combine(res.results)
